# revision 24
# baseline (speedup 1.0000x reference)
# Self-contained Trainium2 Bass kernel for the DetectionSampler module.
# kernel(**inputs) takes the FULL inputs and returns (scores, labels, mask, qlt).
#
# Sharding: data-parallel over batch B=8, one batch per NeuronCore. The
# [Nd,128] distractor set is computed per-core (own batch) and replicated
# across cores with an AllGather collective, per the sharding hint.
import os
import numpy as np

import concourse.bass as bass
import concourse.tile as tile
from concourse import bacc, mybir
from concourse.bass_utils import run_bass_kernel_spmd

AluOp = mybir.AluOpType
AX = mybir.AxisListType
f32, i32, i16 = mybir.dt.float32, mybir.dt.int32, mybir.dt.int16

# ---- problem constants (hardcoded; must match the reference module) ----
B, D, H, W = 8, 128, 256, 256
CELL, BORD = 8, 16
HC = (H - 2 * BORD) // CELL          # 28 cells per side
NPT = HC * HC                        # 784 points per batch
NP112 = 112                          # cell-layout partitions (4*cy + cxhi)
C7 = 7                               # cell-layout free cols (cxlo)
SLOTS = 896                          # 7 * 128 padded gather slots
KP = H * W // 2                      # 32768 pair rows per image
PAIR = 2 * D                         # 256 floats per pair row

POS_R, NEG_MIN_R, NEG_MAX_R, NEG_STEP = 3, 7, 8, 2
_pos = [(i, j) for i in range(-POS_R, POS_R + 1)
        for j in range(-POS_R, POS_R + 1) if i * i + j * j <= POS_R ** 2]
_neg = [(i, j) for i in range(-NEG_MAX_R, NEG_MAX_R + 1, NEG_STEP)
        for j in range(-NEG_MAX_R, NEG_MAX_R + 1, NEG_STEP)
        if NEG_MIN_R ** 2 <= i * i + j * j <= NEG_MAX_R ** 2]
NPOS, NNEG = len(_pos), len(_neg)    # 29, 12
NOFF = NPOS + NNEG                   # 41
# Positive offsets reordered row-major by (dy, dx) so each disc row can be
# gathered as one contiguous pair segment. REFJ maps back to reference order
# (argmax tie-break must follow the reference's offset order).
_pos_rm = sorted(range(NPOS), key=lambda k: (_pos[k][1], _pos[k][0]))
REFJ = np.array([_pos_rm.index(k) for k in range(NPOS)], np.int32)  # myidx of ref k
REFJ = np.argsort(np.array(_pos_rm))  # refj at my position
_pos_s = [_pos[k] for k in _pos_rm]
# xy2 is (x=col, y=row); offsets add (i -> x, j -> y)
DX = np.array([ij[0] for ij in _pos_s + _neg], np.int32)
DY = np.array([ij[1] for ij in _pos_s + _neg], np.int32)
# disc rows: (dy, max|dx|, seg_pairs, col_start, n_offsets_in_row)
ROWS = []
_c = 0
for dy in sorted(set(j for _, j in _pos_s)):
    dxs = [i for i, j in _pos_s if j == dy]
    r = max(abs(min(dxs)), abs(max(dxs)))
    seg = (2 * r + 1) // 2 + 1          # pairs covering 2r+1 pixels, any parity
    ROWS.append((dy, r, seg, _c, len(dxs)))
    _c += len(dxs)
NROWS = len(ROWS)                        # 7
NGL = NROWS + NNEG                       # 19 gather idx lists
NCOL = 1 + NNEG + B * NPT            # 6285 score columns
ND = B * NPT                         # 6272 distractors


def _build_program():
    nc = bacc.Bacc("TRN2", target_bir_lowering=False, debug=False, num_devices=B)

    # ---------------- DRAM I/O ----------------
    dram_in = {}
    def din(name, shape, dt=f32):
        dram_in[name] = nc.dram_tensor(name, shape, dt, kind="ExternalInput").ap()
        return dram_in[name]

    desT1p = din("desT1p", [KP, PAIR])          # own-batch des1, channels-last pairs
    desT2p = din("desT2p", [KP, PAIR])          # own-batch des2, channels-last pairs
    det1c = din("det1c", [NP112, C7 * 64])      # det1 cells [112, 7*64]
    det2c = din("det2c", [NP112, C7 * 64])      # det2 cells (own batch)
    aq = din("aq", [1024, 192])                 # [aflow-x | aflow-y | qlt1] blocks
    posj = din("posj", [64])                    # refj (0:29) and NPOS-refj (32:61)
    qlt2f = din("qlt2f", [1024, 64])            # qlt2 flat, 64-float blocks
    ctab = din("ctab", [2 * 64], i32)           # dx row (0:41), dy row (64:105)

    scores_o = nc.dram_tensor("scores_o", [NPT, NCOL], f32, kind="ExternalOutput").ap()
    qlt_o = nc.dram_tensor("qlt_o", [NPT], f32, kind="ExternalOutput").ap()
    mask_o = nc.dram_tensor("mask_o", [NPT], i32, kind="ExternalOutput").ap()
    ownblk_o = nc.dram_tensor("ownblk_o", [NPT, NPT], f32, kind="ExternalOutput").ap()

    # staging + collective DRAM
    sg_blk1 = nc.dram_tensor("sg_blk1", [SLOTS], i16, kind="Internal").ap()
    sg_lin1 = nc.dram_tensor("sg_lin1", [SLOTS], i32, kind="Internal").ap()
    sg_par1 = nc.dram_tensor("sg_par1", [SLOTS], f32, kind="Internal").ap()
    sg_pidx1 = nc.dram_tensor("sg_pidx1", [SLOTS], i16, kind="Internal").ap()
    sg_goff = nc.dram_tensor("sg_goff", [NGL * SLOTS], i16, kind="Internal").ap()
    sg_qblk = nc.dram_tensor("sg_qblk", [SLOTS], i16, kind="Internal").ap()
    sg_pidx2 = nc.dram_tensor("sg_pidx2", [SLOTS], i16, kind="Internal").ap()
    sg_par2 = nc.dram_tensor("sg_par2", [SLOTS], f32, kind="Internal").ap()
    sg_xyd = nc.dram_tensor("sg_xyd", [2 * SLOTS], f32, kind="Internal").ap()
    distr_in = nc.dram_tensor("distr_in", [D * NPT], f32, kind="Internal").ap()
    distr_all = nc.dram_tensor("distr_all", [B * D * NPT], f32, kind="Internal",
                               addr_space="Shared").ap()

    with tile.TileContext(nc) as tc:
        _emit(nc, tc, dram_in, scores_o, qlt_o, mask_o, ownblk_o,
              sg_blk1, sg_lin1, sg_par1, sg_pidx1, sg_goff, sg_qblk,
              sg_pidx2, sg_par2, sg_xyd, distr_in, distr_all)
    nc.compile()
    return nc


def _emit(nc, tc, din, scores_o, qlt_o, mask_o, ownblk_o,
          sg_blk1, sg_lin1, sg_par1, sg_pidx1, sg_goff, sg_qblk,
          sg_pidx2, sg_par2, sg_xyd, distr_in, distr_all):
    from contextlib import ExitStack
    ctx = ExitStack()
    pool = ctx.enter_context(tc.tile_pool(name="main", bufs=1))
    gpool = ctx.enter_context(tc.tile_pool(name="gath", bufs=2))
    ppool = ctx.enter_context(tc.tile_pool(name="prod", bufs=1))
    opool = ctx.enter_context(tc.tile_pool(name="outc", bufs=3))
    pspool = ctx.enter_context(tc.tile_pool(name="psA", bufs=2, space="PSUM"))
    pspool2 = ctx.enter_context(tc.tile_pool(name="psB", bufs=2, space="PSUM"))

    V, G, S = nc.vector, nc.gpsimd, nc.scalar

    # ---------------- constants ----------------
    ident = pool.tile([128, 128], f32)
    io_p = pool.tile([128, 128], i32)
    io_f = pool.tile([128, 128], i32)
    G.iota(io_p[:], pattern=[[0, 128]], base=0, channel_multiplier=1)
    G.iota(io_f[:], pattern=[[1, 128]], base=0, channel_multiplier=0)
    V.tensor_tensor(ident[:], io_p[:], io_f[:], AluOp.is_equal)

    rev64 = pool.tile([NP112, 64], f32)
    G.iota(rev64[:], pattern=[[-1, 64]], base=64, channel_multiplier=0,
           allow_small_or_imprecise_dtypes=True)
    iot64 = pool.tile([NP112, 64], f32)
    G.iota(iot64[:], pattern=[[1, 64]], base=0, channel_multiplier=0,
           allow_small_or_imprecise_dtypes=True)
    iot64g = pool.tile([128, 64], f32)
    G.iota(iot64g[:], pattern=[[1, 64]], base=0, channel_multiplier=0,
           allow_small_or_imprecise_dtypes=True)
    revP = pool.tile([128, NPOS], f32)
    nc.sync.dma_start(revP[:], bass.AP(din["posj"].tensor, 32, [[0, 128], [1, NPOS]]))
    iotP = pool.tile([128, NPOS], f32)
    nc.sync.dma_start(iotP[:], bass.AP(din["posj"].tensor, 0, [[0, 128], [1, NPOS]]))
    pidx112 = pool.tile([NP112, 1], i32)
    G.iota(pidx112[:], pattern=[[0, 1]], base=0, channel_multiplier=1)
    ci7 = pool.tile([NP112, C7], i32)
    G.iota(ci7[:], pattern=[[1, C7]], base=0, channel_multiplier=0)

    dxg = pool.tile([128, NOFF], i32)
    dyg = pool.tile([128, NOFF], i32)
    nc.sync.dma_start(dxg[:], bass.AP(din["ctab"].tensor, 0, [[0, 128], [1, NOFF]]))
    nc.sync.dma_start(dyg[:], bass.AP(din["ctab"].tensor, 64, [[0, 128], [1, NOFF]]))

    zero16 = pool.tile([1, NP112], i16)
    G.memset(zero16[:], 0)
    zerof = pool.tile([1, NP112], f32)
    G.memset(zerof[:], 0.0)
    zeroi = pool.tile([1, NP112], i32)
    G.memset(zeroi[:], 0)
    pad = lambda t: bass.AP(t.tensor, NPT, [[1, 1], [1, NP112]])
    nc.sync.dma_start(pad(sg_par2), zerof[:])
    nc.sync.dma_start(pad(sg_par1), zerof[:])
    nc.sync.dma_start(bass.AP(sg_blk1.tensor, NPT, [[1, 1], [1, NP112]]), zero16[:])
    nc.sync.dma_start(pad(sg_lin1), zeroi[:])

    # ============ cell pipeline helper: argmax decode on [112, 7, 64] ============
    def cell_argmax(cells_ap, tag):
        ct = pool.tile([NP112, C7, 64], f32, tag="ca_ct")
        nc.sync.dma_start(ct[:], cells_ap.rearrange("p (a b) -> p a b", a=C7))
        mx = pool.tile([NP112, C7], f32, tag=f"mx_{tag}")
        V.tensor_reduce(mx[:], ct[:], axis=AX.X, op=AluOp.max)
        eq = pool.tile([NP112, C7, 64], f32, tag="ca_eq")
        V.tensor_tensor(eq[:], ct[:], mx[:].unsqueeze(2).broadcast_to((NP112, C7, 64)),
                        AluOp.is_equal)
        sc = pool.tile([NP112, C7, 64], f32, tag="ca_sc")
        V.tensor_tensor(sc[:], eq[:], rev64[:].unsqueeze(1).broadcast_to((NP112, C7, 64)),
                        AluOp.mult)
        rs = pool.tile([NP112, C7], f32, tag=f"rs_{tag}")
        V.tensor_reduce(rs[:], sc[:], axis=AX.X, op=AluOp.max)
        jf = pool.tile([NP112, C7], f32, tag=f"jf_{tag}")
        V.tensor_scalar(jf[:], rs[:], 64.0, -1.0, AluOp.subtract, AluOp.mult)
        ji = pool.tile([NP112, C7], i32, tag=f"ji_{tag}")
        V.tensor_copy(ji[:], jf[:])
        # decode: ii=j>>3, jj=j&7; colcoord = 16+8*(7*(p&3)+c)+jj ; rowcoord = 16+8*(p>>2)+ii
        ii = pool.tile([NP112, C7], i32, tag=f"ii_{tag}")
        V.tensor_scalar(ii[:], ji[:], 3, None, AluOp.arith_shift_right)
        jj = pool.tile([NP112, C7], i32, tag=f"jj_{tag}")
        V.tensor_scalar(jj[:], ji[:], 7, None, AluOp.bitwise_and)
        cy = pool.tile([NP112, 1], i32, tag=f"cy_{tag}")
        V.tensor_scalar(cy[:], pidx112[:], 2, None, AluOp.arith_shift_right)
        cxh = pool.tile([NP112, 1], i32, tag=f"cxh_{tag}")
        V.tensor_scalar(cxh[:], pidx112[:], 3, None, AluOp.bitwise_and)
        # rowc = 16 + 8*cy + ii
        rowc = pool.tile([NP112, C7], i32, tag=f"rowc_{tag}")
        t1 = pool.tile([NP112, 1], i32, tag=f"t1_{tag}")
        V.tensor_scalar(t1[:], cy[:], 8, 16, AluOp.mult, AluOp.add)
        V.tensor_tensor(rowc[:], ii[:], t1[:].broadcast_to((NP112, C7)), AluOp.add)
        # colc = 16 + 8*(7*cxh + c) + jj = 16 + 56*cxh + 8*c + jj
        colc = pool.tile([NP112, C7], i32, tag=f"colc_{tag}")
        t2 = pool.tile([NP112, 1], i32, tag=f"t2_{tag}")
        V.tensor_scalar(t2[:], cxh[:], 56, 16, AluOp.mult, AluOp.add)
        t3 = pool.tile([NP112, C7], i32, tag=f"t3_{tag}")
        V.tensor_scalar(t3[:], ci7[:], 8, None, AluOp.mult)
        V.tensor_tensor(t3[:], t3[:], t2[:].broadcast_to((NP112, C7)), AluOp.add)
        V.tensor_tensor(colc[:], jj[:], t3[:], AluOp.add)
        # linear = colc*256 + rowc  (des[., colcoord, rowcoord] per reference swap)
        lin = pool.tile([NP112, C7], i32, tag=f"lin_{tag}")
        V.tensor_scalar(lin[:], colc[:], 256, None, AluOp.mult)
        V.tensor_tensor(lin[:], lin[:], rowc[:], AluOp.add)
        return jf, rowc, colc, lin

    # ================= det2 / distractor path (emit first: feeds collective) ====
    _, rowc2, colc2, lin2 = cell_argmax(din["det2c"], "d2")
    pidx2 = pool.tile([NP112, C7], i32)
    V.tensor_scalar(pidx2[:], lin2[:], 1, None, AluOp.arith_shift_right)
    pidx2_16 = pool.tile([NP112, C7], i16)
    V.tensor_copy(pidx2_16[:], pidx2[:])
    par2 = pool.tile([NP112, C7], i32)
    V.tensor_scalar(par2[:], lin2[:], 1, None, AluOp.bitwise_and)
    par2f = pool.tile([NP112, C7], f32)
    V.tensor_copy(par2f[:], par2[:])
    nc.sync.dma_start(bass.AP(sg_pidx2.tensor, 0, [[C7, NP112], [1, C7]]), pidx2_16[:])
    nc.sync.dma_start(bass.AP(sg_pidx2.tensor, NPT, [[1, 1], [1, NP112]]), zero16[:])
    nc.sync.dma_start(bass.AP(sg_par2.tensor, 0, [[C7, NP112], [1, C7]]), par2f[:])
    # xd (row-decoded) and yd (col-decoded) rows for the distance mask
    rowc2f = pool.tile([NP112, C7], f32)
    V.tensor_copy(rowc2f[:], rowc2[:])
    colc2f = pool.tile([NP112, C7], f32)
    V.tensor_copy(colc2f[:], colc2[:])
    nc.sync.dma_start(bass.AP(sg_xyd.tensor, 0, [[C7, NP112], [1, C7]]), rowc2f[:])
    nc.sync.dma_start(bass.AP(sg_xyd.tensor, SLOTS, [[C7, NP112], [1, C7]]), colc2f[:])

    # wrapped idx readback + pair gather + parity select
    idx2w = pool.tile([128, 56], i16)
    for k in range(8):
        nc.sync.dma_start(idx2w[16 * k:16 * (k + 1), :],
                          bass.AP(sg_pidx2.tensor, 0, [[1, 16], [16, 56]]))
    par2G = pool.tile([128, C7], f32)
    nc.sync.dma_start(par2G[:], bass.AP(sg_par2.tensor, 0, [[1, 128], [128, C7]]))
    par2inv = pool.tile([128, C7], f32)
    V.tensor_scalar(par2inv[:], par2G[:], -1.0, 1.0, AluOp.mult, AluOp.add)
    dpair = pool.tile([128, C7, PAIR], f32, tag="pairg")
    G.dma_gather(dpair[:], din["desT2p"], idx2w[:], num_idxs=SLOTS,
                 num_idxs_reg=SLOTS, elem_size=PAIR)
    dexact = pool.tile([128, C7, D], f32)
    tlo = pool.tile([128, C7, D], f32, tag="seltmp")
    V.tensor_tensor(tlo[:], dpair[:, :, 0:D],
                    par2inv[:].unsqueeze(2).broadcast_to((128, C7, D)), AluOp.mult)
    thi = pool.tile([128, C7, D], f32, tag="seltmp2")
    V.tensor_tensor(thi[:], dpair[:, :, D:PAIR],
                    par2G[:].unsqueeze(2).broadcast_to((128, C7, D)), AluOp.mult)
    V.tensor_tensor(dexact[:], tlo[:], thi[:], AluOp.add)
    # transpose to [desc, slot] and ship own block to the collective
    distrT = pool.tile([128, SLOTS], f32)
    for ci in range(C7):
        pt = pspool2.tile([128, 128], f32, tag="tpose")
        nc.tensor.transpose(pt[:], dexact[:, ci, :], ident[:])
        S.copy(distrT[:, 128 * ci:128 * (ci + 1)], pt[:])
    nc.sync.dma_start(bass.AP(distr_in.tensor, 0, [[NPT, 128], [1, NPT]]),
                      distrT[:, 0:NPT])
    G.collective_compute("AllGather", AluOp.bypass,
                         replica_groups=[list(range(B))],
                         ins=[distr_in], outs=[distr_all])

    # ================= det1 / point path =================
    jf1, rowc1, colc1, lin1 = cell_argmax(din["det1c"], "d1")
    pidx1 = pool.tile([NP112, C7], i32)
    V.tensor_scalar(pidx1[:], lin1[:], 1, None, AluOp.arith_shift_right)
    pidx1_16 = pool.tile([NP112, C7], i16)
    V.tensor_copy(pidx1_16[:], pidx1[:])
    par1 = pool.tile([NP112, C7], i32)
    V.tensor_scalar(par1[:], lin1[:], 1, None, AluOp.bitwise_and)
    par1f = pool.tile([NP112, C7], f32)
    V.tensor_copy(par1f[:], par1[:])
    nc.sync.dma_start(bass.AP(sg_pidx1.tensor, 0, [[C7, NP112], [1, C7]]), pidx1_16[:])
    nc.sync.dma_start(bass.AP(sg_pidx1.tensor, NPT, [[1, 1], [1, NP112]]), zero16[:])
    nc.sync.dma_start(bass.AP(sg_par1.tensor, 0, [[C7, NP112], [1, C7]]), par1f[:])

    # stage the 64-float block index + full linear index of the sample point
    blk1 = pool.tile([NP112, C7], i32)
    V.tensor_scalar(blk1[:], lin1[:], 6, None, AluOp.arith_shift_right)
    blk1_16 = pool.tile([NP112, C7], i16)
    V.tensor_copy(blk1_16[:], blk1[:])
    nc.sync.dma_start(bass.AP(sg_blk1.tensor, 0, [[C7, NP112], [1, C7]]), blk1_16[:])
    nc.sync.dma_start(bass.AP(sg_lin1.tensor, 0, [[C7, NP112], [1, C7]]), lin1[:])

    # G-layout: gather aflow-x/y + qlt1 blocks at lin1 and select elem lin1&63
    blk1w = pool.tile([128, 56], i16)
    for k in range(8):
        nc.sync.dma_start(blk1w[16 * k:16 * (k + 1), :],
                          bass.AP(sg_blk1.tensor, 0, [[1, 16], [16, 56]]))
    lin1G = pool.tile([128, C7], i32)
    nc.sync.dma_start(lin1G[:], bass.AP(sg_lin1.tensor, 0, [[1, 128], [128, C7]]))
    off1 = pool.tile([128, C7], i32)
    V.tensor_scalar(off1[:], lin1G[:], 63, None, AluOp.bitwise_and)
    off1f = pool.tile([128, C7], f32)
    V.tensor_copy(off1f[:], off1[:])
    ohg = pool.tile([128, C7, 64], f32)
    V.tensor_tensor(ohg[:], iot64g[:].unsqueeze(1).broadcast_to((128, C7, 64)),
                    off1f[:].unsqueeze(2).broadcast_to((128, C7, 64)), AluOp.is_equal)

    aqt = pool.tile([128, C7, 3, 64], f32, tag="pairg")
    G.dma_gather(aqt[:].rearrange("p c a b -> p c (a b)"), din["aq"], blk1w[:],
                 num_idxs=SLOTS, num_idxs_reg=SLOTS, elem_size=192)
    V.tensor_tensor(aqt[:], aqt[:], ohg[:].unsqueeze(2).broadcast_to((128, C7, 3, 64)),
                    AluOp.mult)
    aqv = pool.tile([128, C7, 3], f32)
    V.tensor_reduce(aqv[:], aqt[:], axis=AX.X, op=AluOp.add)
    x2f_raw = aqv[:, :, 0]
    y2f_raw = aqv[:, :, 1]
    q1G = aqv[:, :, 2]  # AP slices (not tiles)

    # xy2 = trunc(aflow + 0.5). The f32->i32 cast rounding mode differs between
    # HW (round-nearest-even) and CoreSim (trunc); build trunc explicitly in a
    # mode-agnostic way: floor = i0 - (float(i0) > f); trunc = floor + (f<0 & f!=floor)
    def trunc_to_int(raw_ap, tag):
        f = pool.tile([128, C7], f32, tag=f"tr_f_{tag}")
        V.tensor_scalar(f[:], raw_ap, 0.5, None, AluOp.add)
        i0 = pool.tile([128, C7], i32, tag=f"tr_i0_{tag}")
        V.tensor_copy(i0[:], f[:])
        fi = pool.tile([128, C7], f32, tag=f"tr_fi_{tag}")
        V.tensor_copy(fi[:], i0[:])
        gt = pool.tile([128, C7], f32, tag=f"tr_gt_{tag}")
        V.tensor_tensor(gt[:], fi[:], f[:], AluOp.is_gt)
        gti = pool.tile([128, C7], i32, tag=f"tr_gti_{tag}")
        V.tensor_copy(gti[:], gt[:])
        fl = pool.tile([128, C7], i32, tag=f"tr_fl_{tag}")
        V.tensor_tensor(fl[:], i0[:], gti[:], AluOp.subtract)
        flf = pool.tile([128, C7], f32, tag=f"tr_flf_{tag}")
        V.tensor_copy(flf[:], fl[:])
        neg = pool.tile([128, C7], f32, tag=f"tr_neg_{tag}")
        V.tensor_scalar(neg[:], f[:], 0.0, None, AluOp.is_lt)
        ne = pool.tile([128, C7], f32, tag=f"tr_ne_{tag}")
        V.tensor_tensor(ne[:], flf[:], f[:], AluOp.not_equal)
        V.tensor_tensor(neg[:], neg[:], ne[:], AluOp.mult)
        negi = pool.tile([128, C7], i32, tag=f"tr_negi_{tag}")
        V.tensor_copy(negi[:], neg[:])
        out = pool.tile([128, C7], i32, tag=f"tr_out_{tag}")
        V.tensor_tensor(out[:], fl[:], negi[:], AluOp.add)
        return out

    x2G = trunc_to_int(x2f_raw, "x")
    y2G = trunc_to_int(y2f_raw, "y")

    # validity mask (G-layout; write with c-split to stay in [0,784))
    m1 = pool.tile([128, C7], i32, tag="m1")
    m2 = pool.tile([128, C7], i32, tag="m2")
    V.tensor_scalar(m1[:], x2G[:], 0, None, AluOp.is_ge)
    V.tensor_scalar(m2[:], x2G[:], W, None, AluOp.is_lt)
    V.tensor_tensor(m1[:], m1[:], m2[:], AluOp.mult)
    V.tensor_scalar(m2[:], y2G[:], 0, None, AluOp.is_ge)
    V.tensor_tensor(m1[:], m1[:], m2[:], AluOp.mult)
    V.tensor_scalar(m2[:], y2G[:], H, None, AluOp.is_lt)
    V.tensor_tensor(m1[:], m1[:], m2[:], AluOp.mult)
    nc.sync.dma_start(bass.AP(mask_o.tensor, 0, [[1, 128], [128, 6]]), m1[:, 0:6])
    nc.sync.dma_start(bass.AP(mask_o.tensor, 768, [[1, 16], [1, 1]]), m1[0:16, 6:7])

    # ============ G-layout readbacks ============
    par1G = pool.tile([128, C7], f32)
    nc.sync.dma_start(par1G[:], bass.AP(sg_par1.tensor, 0, [[1, 128], [128, C7]]))
    par1inv = pool.tile([128, C7], f32)
    V.tensor_scalar(par1inv[:], par1G[:], -1.0, 1.0, AluOp.mult, AluOp.add)
    idx1w = pool.tile([128, 56], i16)
    for k in range(8):
        nc.sync.dma_start(idx1w[16 * k:16 * (k + 1), :],
                          bass.AP(sg_pidx1.tensor, 0, [[1, 16], [16, 56]]))
    xdR = pool.tile([128, NPT], f32)
    nc.sync.dma_start(xdR[:], bass.AP(sg_xyd.tensor, 0, [[0, 128], [1, NPT]]))
    ydR = pool.tile([128, NPT], f32)
    nc.sync.dma_start(ydR[:], bass.AP(sg_xyd.tensor, SLOTS, [[0, 128], [1, NPT]]))

    # per-offset clamped coords, parity, pair index (all in G-layout)
    xc = pool.tile([128, C7, NOFF], i32)
    V.tensor_tensor(xc[:], x2G[:].unsqueeze(2).broadcast_to((128, C7, NOFF)),
                    dxg[:].unsqueeze(1).broadcast_to((128, C7, NOFF)), AluOp.add)
    V.tensor_scalar(xc[:], xc[:], 0, W - 1, AluOp.max, AluOp.min)
    yc = pool.tile([128, C7, NOFF], i32)
    V.tensor_tensor(yc[:], y2G[:].unsqueeze(2).broadcast_to((128, C7, NOFF)),
                    dyg[:].unsqueeze(1).broadcast_to((128, C7, NOFF)), AluOp.add)
    V.tensor_scalar(yc[:], yc[:], 0, H - 1, AluOp.max, AluOp.min)
    parO = pool.tile([128, C7, NOFF], f32)
    tpo = pool.tile([128, C7, NOFF], i32, tag="tpo")
    V.tensor_scalar(tpo[:], xc[:], 1, None, AluOp.bitwise_and)
    V.tensor_copy(parO[:], tpo[:])
    parOinv = pool.tile([128, C7, NOFF], f32)
    V.tensor_scalar(parOinv[:], parO[:], -1.0, 1.0, AluOp.mult, AluOp.add)
    pidxO = pool.tile([128, C7, NOFF], i32)
    V.tensor_scalar(pidxO[:], xc[:], 1, None, AluOp.arith_shift_right)
    tyo = pool.tile([128, C7, NOFF], i32, tag="tyo")
    V.tensor_scalar(tyo[:], yc[:], W // 2, None, AluOp.mult)
    V.tensor_tensor(pidxO[:], pidxO[:], tyo[:], AluOp.add)
    V.tensor_scalar(pidxO[:], pidxO[:], 0, KP - 1, AluOp.max, AluOp.min)
    # gather idx lists: 7 pos row-segment starts + 12 neg pair indices
    gidx = pool.tile([128, NGL, C7], i32)
    k0all = pool.tile([128, NROWS, C7], i32)   # row segment start pairs
    for ri, (dy, r, seg, c0, nw) in enumerate(ROWS):
        ycr = pool.tile([128, C7], i32, tag="ycr")
        V.tensor_scalar(ycr[:], y2G[:], dy, None, AluOp.add)
        V.tensor_scalar(ycr[:], ycr[:], 0, H - 1, AluOp.max, AluOp.min)
        k0 = pool.tile([128, C7], i32, tag="k0r")
        V.tensor_scalar(k0[:], x2G[:], r, None, AluOp.subtract)
        V.tensor_scalar(k0[:], k0[:], 1, None, AluOp.arith_shift_right)
        V.tensor_scalar(k0[:], k0[:], 0, W // 2 - seg, AluOp.max, AluOp.min)
        V.tensor_copy(k0all[:, ri, :], k0[:])
        V.tensor_scalar(ycr[:], ycr[:], W // 2, None, AluOp.mult)
        V.tensor_tensor(gidx[:, ri, :], ycr[:], k0[:], AluOp.add)
    V.tensor_copy(gidx[:, NROWS:NGL, :],
                  pidxO[:, :, NPOS:NOFF].rearrange("p c o -> p o c"))
    gidx16 = pool.tile([128, NGL, C7], i16)
    V.tensor_copy(gidx16[:], gidx[:])
    nc.sync.dma_start(bass.AP(sg_goff.tensor, 0,
                              [[1, 128], [SLOTS, NGL], [128, C7]]), gidx16[:])
    goffw = pool.tile([128, NGL, 56], i16)
    for k in range(8):
        nc.sync.dma_start(goffw[16 * k:16 * (k + 1), :, :],
                          bass.AP(sg_goff.tensor, 0, [[1, 16], [SLOTS, NGL], [16, 56]]))

    # s_des1 gather + parity select
    spair = pool.tile([128, C7, PAIR], f32, tag="pairg")
    G.dma_gather(spair[:], din["desT1p"], idx1w[:], num_idxs=SLOTS,
                 num_idxs_reg=SLOTS, elem_size=PAIR)
    sexact = pool.tile([128, C7, D], f32)
    tl1 = pool.tile([128, C7, D], f32, tag="seltmp")
    V.tensor_tensor(tl1[:], spair[:, :, 0:D],
                    par1inv[:].unsqueeze(2).broadcast_to((128, C7, D)), AluOp.mult)
    th1 = pool.tile([128, C7, D], f32, tag="seltmp2")
    V.tensor_tensor(th1[:], spair[:, :, D:PAIR],
                    par1G[:].unsqueeze(2).broadcast_to((128, C7, D)), AluOp.mult)
    V.tensor_tensor(sexact[:], tl1[:], th1[:], AluOp.add)

    # ============ sT transposes ============
    sT = pool.tile([128, SLOTS], f32)
    for ci in range(C7):
        pt = pspool2.tile([128, 128], f32, tag="tpose")
        nc.tensor.transpose(pt[:], sexact[:, ci, :], ident[:])
        S.copy(sT[:, 128 * ci:128 * (ci + 1)], pt[:])

    # ============ distractor GEMM (global, unmasked) ============
    CH = 512
    for bb in range(B):
        rhs = pool.tile([128, NPT], f32, tag="rhs")
        nc.sync.dma_start(rhs[:], bass.AP(distr_all.tensor, bb * D * NPT,
                                          [[NPT, 128], [1, NPT]]))
        for ci in range(C7):
            rows = 128 if ci < 6 else 16
            for ch in range(2):
                c0 = ch * CH
                cw = min(CH, NPT - c0)
                pm = pspool.tile([128, CH], f32, tag="gemm")
                nc.tensor.matmul(pm[:, 0:cw], sT[:, 128 * ci:128 * (ci + 1)],
                                 rhs[:, c0:c0 + cw], start=True, stop=True)
                ob = opool.tile([128, CH], f32, tag="ochunk")
                S.copy(ob[:, 0:cw], pm[:, 0:cw])
                nc.sync.dma_start(
                    bass.AP(scores_o.tensor,
                            128 * ci * NCOL + 1 + NNEG + bb * NPT + c0,
                            [[NCOL, rows], [1, cw]]), ob[0:rows, 0:cw])

    # ============ own-block GEMM + distance mask (static; host overlays) ====
    for ci in range(C7):
        rows = 128 if ci < 6 else 16
        # dis2 = (xd - x2)^2 + (yd - y2)^2 over own 784 cols
        x2s = pool.tile([128, 1], f32, tag="x2s")
        V.tensor_copy(x2s[:], x2G[:, ci:ci + 1])
        y2s = pool.tile([128, 1], f32, tag="y2s")
        V.tensor_copy(y2s[:], y2G[:, ci:ci + 1])
        dxm = pool.tile([128, NPT], f32, tag="dxm")
        V.tensor_tensor(dxm[:], xdR[:], x2s[:].broadcast_to((128, NPT)), AluOp.subtract)
        V.tensor_tensor(dxm[:], dxm[:], dxm[:], AluOp.mult)
        dym = pool.tile([128, NPT], f32, tag="dym")
        V.tensor_tensor(dym[:], ydR[:], y2s[:].broadcast_to((128, NPT)), AluOp.subtract)
        V.tensor_tensor(dym[:], dym[:], dym[:], AluOp.mult)
        V.tensor_tensor(dxm[:], dxm[:], dym[:], AluOp.add)
        keep = pool.tile([128, NPT], f32, tag="keep")
        V.tensor_scalar(keep[:], dxm[:], float(POS_R ** 2), None, AluOp.is_ge)
        for ch in range(2):
            c0 = ch * CH
            cw = min(CH, NPT - c0)
            pm = pspool.tile([128, CH], f32, tag="gemm")
            nc.tensor.matmul(pm[:, 0:cw], sT[:, 128 * ci:128 * (ci + 1)],
                             distrT[:, c0:c0 + cw], start=True, stop=True)
            ob = opool.tile([128, CH], f32, tag="ochunk")
            S.copy(ob[:, 0:cw], pm[:, 0:cw])
            V.tensor_tensor(ob[:, 0:cw], ob[:, 0:cw], keep[:, c0:c0 + cw], AluOp.mult)
            nc.sync.dma_start(
                bass.AP(ownblk_o.tensor, 128 * ci * NPT + c0, [[NPT, rows], [1, cw]]),
                ob[0:rows, 0:cw])

    # ============ dot computation ============
    dotsAll = pool.tile([128, C7, NOFF], f32)
    # pos rows: gather SEG-pair segments, per-pixel dots, select by position
    for ri, (dy, r, seg, c0, nw) in enumerate(ROWS):
        npx = 2 * seg
        gp = gpool.tile([128, C7, seg * PAIR], f32, tag="gseg")
        seg_src = bass.AP(din["desT2p"].tensor, 0,
                          [[PAIR, KP - seg + 1], [1, seg * PAIR]])
        G.dma_gather(gp[:], seg_src, goffw[:, ri, :], num_idxs=SLOTS,
                     num_idxs_reg=SLOTS, elem_size=seg * PAIR, elem_step=PAIR)
        pr = ppool.tile([128, C7, npx, D], f32, tag="pr")
        V.tensor_tensor(pr[:], gp[:].rearrange("p c (h d) -> p c h d", h=npx),
                        sexact[:].unsqueeze(2).broadcast_to((128, C7, npx, D)),
                        AluOp.mult)
        pxd = pool.tile([128, C7, npx], f32, tag="pxd")
        V.tensor_reduce(pxd[:], pr[:], axis=AX.X, op=AluOp.add)
        # position of each offset's pixel inside the segment: xc - 2*k0
        posr = pool.tile([128, C7, nw], f32, tag="posr")
        k2 = pool.tile([128, C7], i32, tag="k2r")
        V.tensor_scalar(k2[:], k0all[:, ri, :], 2, None, AluOp.mult)
        posi = pool.tile([128, C7, nw], i32, tag="posi")
        V.tensor_tensor(posi[:], xc[:, :, c0:c0 + nw],
                        k2[:].unsqueeze(2).broadcast_to((128, C7, nw)), AluOp.subtract)
        V.tensor_copy(posr[:], posi[:])
        iopx = pool.tile([128, npx], f32, tag="iopx")
        G.iota(iopx[:], pattern=[[1, npx]], base=0, channel_multiplier=0,
               allow_small_or_imprecise_dtypes=True)
        ohr = ppool.tile([128, C7, nw, npx], f32, tag="ohr")
        V.tensor_tensor(ohr[:],
                        iopx[:].unsqueeze(1).unsqueeze(2).broadcast_to((128, C7, nw, npx)),
                        posr[:].unsqueeze(3).broadcast_to((128, C7, nw, npx)),
                        AluOp.is_equal)
        V.tensor_tensor(ohr[:], ohr[:],
                        pxd[:].unsqueeze(2).broadcast_to((128, C7, nw, npx)), AluOp.mult)
        V.tensor_reduce(dotsAll[:, :, c0:c0 + nw], ohr[:], axis=AX.X, op=AluOp.add)
    # neg offsets: per-offset pair gathers + parity blend
    dots2All = pool.tile([128, C7, NNEG, 2], f32)
    for o in range(NNEG):
        gp = gpool.tile([128, C7, PAIR], f32, tag="gseg")
        G.dma_gather(gp[:], din["desT2p"], goffw[:, NROWS + o, :], num_idxs=SLOTS,
                     num_idxs_reg=SLOTS, elem_size=PAIR)
        pr = ppool.tile([128, C7, 2, D], f32, tag="pr")
        V.tensor_tensor(pr[:], gp[:].rearrange("p c (h d) -> p c h d", h=2),
                        sexact[:].unsqueeze(2).broadcast_to((128, C7, 2, D)),
                        AluOp.mult)
        V.tensor_reduce(dots2All[:, :, o, :], pr[:], axis=AX.X, op=AluOp.add)
    b0 = pool.tile([128, C7, NNEG], f32, tag="blend0")
    V.tensor_tensor(b0[:], dots2All[:, :, :, 0], parOinv[:, :, NPOS:NOFF], AluOp.mult)
    b1 = pool.tile([128, C7, NNEG], f32, tag="blend1")
    V.tensor_tensor(b1[:], dots2All[:, :, :, 1], parO[:, :, NPOS:NOFF], AluOp.mult)
    V.tensor_tensor(dotsAll[:, :, NPOS:NOFF], b0[:], b1[:], AluOp.add)

    # pscores / first-argmax over the 29 positive offsets
    psc = pool.tile([128, C7], f32)
    V.tensor_reduce(psc[:], dotsAll[:, :, 0:NPOS], axis=AX.X, op=AluOp.max)
    eqp = pool.tile([128, C7, NPOS], f32)
    V.tensor_tensor(eqp[:], dotsAll[:, :, 0:NPOS],
                    psc[:].unsqueeze(2).broadcast_to((128, C7, NPOS)), AluOp.is_equal)
    scp = pool.tile([128, C7, NPOS], f32)
    V.tensor_tensor(scp[:], eqp[:], revP[:].unsqueeze(1).broadcast_to((128, C7, NPOS)),
                    AluOp.mult)
    rsp = pool.tile([128, C7], f32)
    V.tensor_reduce(rsp[:], scp[:], axis=AX.X, op=AluOp.max)
    jsel = pool.tile([128, C7], f32)
    V.tensor_scalar(jsel[:], rsp[:], float(NPOS), -1.0, AluOp.subtract, AluOp.mult)
    ohp = pool.tile([128, C7, NPOS], f32)
    V.tensor_tensor(ohp[:], iotP[:].unsqueeze(1).broadcast_to((128, C7, NPOS)),
                    jsel[:].unsqueeze(2).broadcast_to((128, C7, NPOS)), AluOp.is_equal)
    xcf = pool.tile([128, C7, NPOS], f32)
    V.tensor_copy(xcf[:], xc[:, :, 0:NPOS])
    ycf = pool.tile([128, C7, NPOS], f32)
    V.tensor_copy(ycf[:], yc[:, :, 0:NPOS])
    tsel = pool.tile([128, C7, NPOS], f32, tag="tsel")
    V.tensor_tensor(tsel[:], ohp[:], xcf[:], AluOp.mult)
    selx = pool.tile([128, C7], f32)
    V.tensor_reduce(selx[:], tsel[:], axis=AX.X, op=AluOp.add)
    V.tensor_tensor(tsel[:], ohp[:], ycf[:], AluOp.mult)
    sely = pool.tile([128, C7], f32)
    V.tensor_reduce(sely[:], tsel[:], axis=AX.X, op=AluOp.add)
    # qlt2 block gather: linear = sely*W + selx ; block = lin>>6 ; off = lin&63
    slin = pool.tile([128, C7], f32, tag="slin")
    V.tensor_scalar(slin[:], sely[:], float(W), None, AluOp.mult)
    V.tensor_tensor(slin[:], slin[:], selx[:], AluOp.add)
    slini = pool.tile([128, C7], i32)
    V.tensor_copy(slini[:], slin[:])
    qblk = pool.tile([128, C7], i32)
    V.tensor_scalar(qblk[:], slini[:], 6, None, AluOp.arith_shift_right)
    V.tensor_scalar(qblk[:], qblk[:], 0, 1023, AluOp.max, AluOp.min)
    qoff = pool.tile([128, C7], i32)
    V.tensor_scalar(qoff[:], slini[:], 63, None, AluOp.bitwise_and)
    qofff = pool.tile([128, C7], f32)
    V.tensor_copy(qofff[:], qoff[:])
    qblk16 = pool.tile([128, C7], i16)
    V.tensor_copy(qblk16[:], qblk[:])
    nc.sync.dma_start(bass.AP(sg_qblk.tensor, 0, [[1, 128], [128, C7]]), qblk16[:])
    qblkw = pool.tile([128, 56], i16)
    for k in range(8):
        nc.sync.dma_start(qblkw[16 * k:16 * (k + 1), :],
                          bass.AP(sg_qblk.tensor, 0, [[1, 16], [16, 56]]))
    q2t = pool.tile([128, C7, 64], f32)
    G.dma_gather(q2t[:], din["qlt2f"], qblkw[:], num_idxs=SLOTS,
                 num_idxs_reg=SLOTS, elem_size=64)
    oh64 = pool.tile([128, C7, 64], f32)
    V.tensor_tensor(oh64[:], iot64g[:].unsqueeze(1).broadcast_to((128, C7, 64)),
                    qofff[:].unsqueeze(2).broadcast_to((128, C7, 64)), AluOp.is_equal)
    V.tensor_tensor(q2t[:], q2t[:], oh64[:], AluOp.mult)
    q2v = pool.tile([128, C7], f32)
    V.tensor_reduce(q2v[:], q2t[:], axis=AX.X, op=AluOp.add)
    qfin = pool.tile([128, C7], f32)
    V.tensor_tensor(qfin[:], q1G, q2v[:], AluOp.add)
    V.tensor_scalar(qfin[:], qfin[:], 0.5, None, AluOp.mult)
    # outputs: qlt, pscores, nscores (split c=0..5 full, c=6 first 16 rows)
    nc.sync.dma_start(bass.AP(qlt_o.tensor, 0, [[1, 128], [128, 6]]), qfin[:, 0:6])
    nc.sync.dma_start(bass.AP(qlt_o.tensor, 768, [[1, 16], [1, 1]]), qfin[0:16, 6:7])
    nc.sync.dma_start(bass.AP(scores_o.tensor, 0, [[NCOL, 128], [128 * NCOL, 6]]),
                      psc[:, 0:6])
    nc.sync.dma_start(bass.AP(scores_o.tensor, 768 * NCOL, [[NCOL, 16], [1, 1]]),
                      psc[0:16, 6:7])
    nc.sync.dma_start(bass.AP(scores_o.tensor, 1, [[NCOL, 128], [128 * NCOL, 6], [1, NNEG]]),
                      dotsAll[:, 0:6, NPOS:NOFF])
    nc.sync.dma_start(bass.AP(scores_o.tensor, 768 * NCOL + 1, [[NCOL, 16], [1, NNEG]]),
                      dotsAll[0:16, 6, NPOS:NOFF])

    ctx.close()


# ---------------- host side ----------------
_NC = None

def _get_nc():
    global _NC
    if _NC is None:
        _NC = _build_program()
    return _NC


def _cells(img):
    # [256,256] -> [112, 448] cell layout: p=4*cy+cxhi, f=cxlo*64+ii*8+jj
    a = img[BORD:H - BORD, BORD:W - BORD]
    a = a.reshape(HC, CELL, HC, CELL).transpose(0, 2, 1, 3)      # [cy, cx, ii, jj]
    a = a.reshape(HC, 4, C7, CELL, CELL)                          # [cy, cxhi, cxlo, ii, jj]
    return np.ascontiguousarray(a.reshape(NP112, C7 * 64))


def _make_in_maps(des1, det1, qlt1, des2, det2, qlt2, aflow):
    ctab = np.zeros(2 * 64, np.int32)
    ctab[0:NOFF] = DX
    ctab[64:64 + NOFF] = DY
    posj = np.zeros(64, np.float32)
    posj[0:NPOS] = REFJ.astype(np.float32)
    posj[32:32 + NPOS] = float(NPOS) - REFJ.astype(np.float32)
    in_maps = []
    for b in range(B):
        d1 = np.ascontiguousarray(des1[b].transpose(1, 2, 0)).reshape(KP, PAIR)
        d2 = np.ascontiguousarray(des2[b].transpose(1, 2, 0)).reshape(KP, PAIR)
        in_maps.append({
            "desT1p": d1,
            "desT2p": d2,
            "det1c": _cells(det1[b, 0]),
            "det2c": _cells(det2[b, 0]),
            "aq": np.ascontiguousarray(np.concatenate(
                [aflow[b, 0].reshape(1024, 64), aflow[b, 1].reshape(1024, 64),
                 qlt1[b, 0].reshape(1024, 64)], axis=1)),
            "posj": posj,
            "qlt2f": np.ascontiguousarray(qlt2[b, 0].reshape(1024, 64)),
            "ctab": ctab,
        })
    return in_maps


def _assemble(results):
    scores = np.empty((ND, NCOL), np.float32)
    qlt = np.empty((ND, 1), np.float32)
    mask = np.empty((B, NPT), bool)
    for b in range(B):
        r = results[b]
        rows = slice(b * NPT, (b + 1) * NPT)
        scores[rows] = r["scores_o"]
        # overlay the masked own-batch distractor block
        scores[rows, 1 + NNEG + b * NPT: 1 + NNEG + (b + 1) * NPT] = r["ownblk_o"]
        qlt[rows, 0] = r["qlt_o"]
        mask[b] = r["mask_o"].astype(bool)
    labels = np.zeros((ND, NCOL), bool)
    labels[:, :1] = True
    return scores, labels, mask, qlt


def kernel(des1, det1, qlt1, des2, det2, qlt2, aflow):
    des1 = np.asarray(des1); det1 = np.asarray(det1); qlt1 = np.asarray(qlt1)
    des2 = np.asarray(des2); det2 = np.asarray(det2); qlt2 = np.asarray(qlt2)
    aflow = np.asarray(aflow)
    in_maps = _make_in_maps(des1, det1, qlt1, des2, det2, qlt2, aflow)
    nc = _get_nc()
    trace = os.environ.get("KERNEL_TRACE") == "1"
    res = run_bass_kernel_spmd(nc, in_maps, core_ids=list(range(B)), trace=trace)
    if trace:
        kernel.last_exec_ns = res.exec_time_ns
    return _assemble(res.results)


kernel.last_exec_ns = None


# revision 26
# speedup vs baseline: 1.0828x; 1.0828x over previous
# Self-contained Trainium2 Bass kernel for the DetectionSampler module.
# kernel(**inputs) takes the FULL inputs and returns (scores, labels, mask, qlt).
#
# Sharding: data-parallel over batch B=8, one batch per NeuronCore. The
# [Nd,128] distractor set is computed per-core (own batch) and replicated
# across cores with an AllGather collective, per the sharding hint.
import os
import numpy as np

import concourse.bass as bass
import concourse.tile as tile
from concourse import bacc, mybir
from concourse.bass_utils import run_bass_kernel_spmd

AluOp = mybir.AluOpType
AX = mybir.AxisListType
f32, i32, i16 = mybir.dt.float32, mybir.dt.int32, mybir.dt.int16

# ---- problem constants (hardcoded; must match the reference module) ----
B, D, H, W = 8, 128, 256, 256
CELL, BORD = 8, 16
HC = (H - 2 * BORD) // CELL          # 28 cells per side
NPT = HC * HC                        # 784 points per batch
NP112 = 112                          # cell-layout partitions (4*cy + cxhi)
C7 = 7                               # cell-layout free cols (cxlo)
SLOTS = 896                          # 7 * 128 padded gather slots
KP = H * W // 2                      # 32768 pair rows per image
PAIR = 2 * D                         # 256 floats per pair row

POS_R, NEG_MIN_R, NEG_MAX_R, NEG_STEP = 3, 7, 8, 2
_pos = [(i, j) for i in range(-POS_R, POS_R + 1)
        for j in range(-POS_R, POS_R + 1) if i * i + j * j <= POS_R ** 2]
_neg = [(i, j) for i in range(-NEG_MAX_R, NEG_MAX_R + 1, NEG_STEP)
        for j in range(-NEG_MAX_R, NEG_MAX_R + 1, NEG_STEP)
        if NEG_MIN_R ** 2 <= i * i + j * j <= NEG_MAX_R ** 2]
NPOS, NNEG = len(_pos), len(_neg)    # 29, 12
NOFF = NPOS + NNEG                   # 41
# Positive offsets reordered row-major by (dy, dx) so each disc row can be
# gathered as one contiguous pair segment. REFJ maps back to reference order
# (argmax tie-break must follow the reference's offset order).
_pos_rm = sorted(range(NPOS), key=lambda k: (_pos[k][1], _pos[k][0]))
REFJ = np.array([_pos_rm.index(k) for k in range(NPOS)], np.int32)  # myidx of ref k
REFJ = np.argsort(np.array(_pos_rm))  # refj at my position
_pos_s = [_pos[k] for k in _pos_rm]
# xy2 is (x=col, y=row); offsets add (i -> x, j -> y)
DX = np.array([ij[0] for ij in _pos_s + _neg], np.int32)
DY = np.array([ij[1] for ij in _pos_s + _neg], np.int32)
# disc rows: (dy, max|dx|, seg_pairs, col_start, n_offsets_in_row)
ROWS = []
_c = 0
for dy in sorted(set(j for _, j in _pos_s)):
    dxs = [i for i, j in _pos_s if j == dy]
    r = max(abs(min(dxs)), abs(max(dxs)))
    seg = (2 * r + 1) // 2 + 1          # pairs covering 2r+1 pixels, any parity
    ROWS.append((dy, r, seg, _c, len(dxs)))
    _c += len(dxs)
NROWS = len(ROWS)                        # 7
NGL = NROWS + NNEG                       # 19 gather idx lists
NCOL = 1 + NNEG + B * NPT            # 6285 score columns
ND = B * NPT                         # 6272 distractors


def _build_program():
    nc = bacc.Bacc("TRN2", target_bir_lowering=False, debug=False, num_devices=B)

    # ---------------- DRAM I/O ----------------
    dram_in = {}
    def din(name, shape, dt=f32):
        dram_in[name] = nc.dram_tensor(name, shape, dt, kind="ExternalInput").ap()
        return dram_in[name]

    desT1p = din("desT1p", [KP, PAIR])          # own-batch des1, channels-last pairs
    desT2p = din("desT2p", [KP, PAIR])          # own-batch des2, channels-last pairs
    det1c = din("det1c", [NP112, C7 * 64])      # det1 cells [112, 7*64]
    det2c = din("det2c", [NP112, C7 * 64])      # det2 cells (own batch)
    aq = din("aq", [1024, 192])                 # [aflow-x | aflow-y | qlt1] blocks
    posj = din("posj", [64])                    # refj (0:29) and NPOS-refj (32:61)
    qlt2f = din("qlt2f", [1024, 64])            # qlt2 flat, 64-float blocks
    ctab = din("ctab", [2 * 64], i32)           # dx row (0:41), dy row (64:105)

    scores_o = nc.dram_tensor("scores_o", [NPT, NCOL], f32, kind="ExternalOutput").ap()
    qlt_o = nc.dram_tensor("qlt_o", [NPT], f32, kind="ExternalOutput").ap()
    mask_o = nc.dram_tensor("mask_o", [NPT], i32, kind="ExternalOutput").ap()
    ownblk_o = nc.dram_tensor("ownblk_o", [NPT, NPT], f32, kind="ExternalOutput").ap()

    # staging + collective DRAM
    sg_blk1 = nc.dram_tensor("sg_blk1", [SLOTS], i16, kind="Internal").ap()
    sg_lin1 = nc.dram_tensor("sg_lin1", [SLOTS], i32, kind="Internal").ap()
    sg_par1 = nc.dram_tensor("sg_par1", [SLOTS], f32, kind="Internal").ap()
    sg_pidx1 = nc.dram_tensor("sg_pidx1", [SLOTS], i16, kind="Internal").ap()
    sg_goff = nc.dram_tensor("sg_goff", [NGL * SLOTS], i16, kind="Internal").ap()
    sg_qblk = nc.dram_tensor("sg_qblk", [SLOTS], i16, kind="Internal").ap()
    sg_pidx2 = nc.dram_tensor("sg_pidx2", [SLOTS], i16, kind="Internal").ap()
    sg_par2 = nc.dram_tensor("sg_par2", [SLOTS], f32, kind="Internal").ap()
    sg_xyd = nc.dram_tensor("sg_xyd", [2 * SLOTS], f32, kind="Internal").ap()
    distr_in = nc.dram_tensor("distr_in", [D * NPT], f32, kind="Internal").ap()
    distr_all = nc.dram_tensor("distr_all", [B * D * NPT], f32, kind="Internal",
                               addr_space="Shared").ap()

    with tile.TileContext(nc) as tc:
        _emit(nc, tc, dram_in, scores_o, qlt_o, mask_o, ownblk_o,
              sg_blk1, sg_lin1, sg_par1, sg_pidx1, sg_goff, sg_qblk,
              sg_pidx2, sg_par2, sg_xyd, distr_in, distr_all)
    nc.compile()
    return nc


def _emit(nc, tc, din, scores_o, qlt_o, mask_o, ownblk_o,
          sg_blk1, sg_lin1, sg_par1, sg_pidx1, sg_goff, sg_qblk,
          sg_pidx2, sg_par2, sg_xyd, distr_in, distr_all):
    from contextlib import ExitStack
    ctx = ExitStack()
    pool = ctx.enter_context(tc.tile_pool(name="main", bufs=1))
    gpool = ctx.enter_context(tc.tile_pool(name="gath", bufs=2))
    ppool = ctx.enter_context(tc.tile_pool(name="prod", bufs=1))
    opool = ctx.enter_context(tc.tile_pool(name="outc", bufs=3))
    pspool = ctx.enter_context(tc.tile_pool(name="psA", bufs=2, space="PSUM"))
    pspool2 = ctx.enter_context(tc.tile_pool(name="psB", bufs=2, space="PSUM"))

    V, G, S = nc.vector, nc.gpsimd, nc.scalar

    # ---------------- constants ----------------
    ident = pool.tile([128, 128], f32)
    io_p = pool.tile([128, 128], i32)
    io_f = pool.tile([128, 128], i32)
    G.iota(io_p[:], pattern=[[0, 128]], base=0, channel_multiplier=1)
    G.iota(io_f[:], pattern=[[1, 128]], base=0, channel_multiplier=0)
    V.tensor_tensor(ident[:], io_p[:], io_f[:], AluOp.is_equal)

    rev64 = pool.tile([NP112, 64], f32)
    G.iota(rev64[:], pattern=[[-1, 64]], base=64, channel_multiplier=0,
           allow_small_or_imprecise_dtypes=True)
    iot64 = pool.tile([NP112, 64], f32)
    G.iota(iot64[:], pattern=[[1, 64]], base=0, channel_multiplier=0,
           allow_small_or_imprecise_dtypes=True)
    iot64g = pool.tile([128, 64], f32)
    G.iota(iot64g[:], pattern=[[1, 64]], base=0, channel_multiplier=0,
           allow_small_or_imprecise_dtypes=True)
    revP = pool.tile([128, NPOS], f32)
    nc.sync.dma_start(revP[:], bass.AP(din["posj"].tensor, 32, [[0, 128], [1, NPOS]]))
    iotP = pool.tile([128, NPOS], f32)
    nc.sync.dma_start(iotP[:], bass.AP(din["posj"].tensor, 0, [[0, 128], [1, NPOS]]))
    pidx112 = pool.tile([NP112, 1], i32)
    G.iota(pidx112[:], pattern=[[0, 1]], base=0, channel_multiplier=1)
    ci7 = pool.tile([NP112, C7], i32)
    G.iota(ci7[:], pattern=[[1, C7]], base=0, channel_multiplier=0)

    dxg = pool.tile([128, NOFF], i32)
    dyg = pool.tile([128, NOFF], i32)
    nc.sync.dma_start(dxg[:], bass.AP(din["ctab"].tensor, 0, [[0, 128], [1, NOFF]]))
    nc.sync.dma_start(dyg[:], bass.AP(din["ctab"].tensor, 64, [[0, 128], [1, NOFF]]))

    zero16 = pool.tile([1, NP112], i16)
    G.memset(zero16[:], 0)
    zerof = pool.tile([1, NP112], f32)
    G.memset(zerof[:], 0.0)
    zeroi = pool.tile([1, NP112], i32)
    G.memset(zeroi[:], 0)
    pad = lambda t: bass.AP(t.tensor, NPT, [[1, 1], [1, NP112]])
    nc.sync.dma_start(pad(sg_par2), zerof[:])
    nc.sync.dma_start(pad(sg_par1), zerof[:])
    nc.sync.dma_start(pad(sg_lin1), zeroi[:])

    # ============ cell pipeline helper: argmax decode on [112, 7, 64] ============
    def cell_argmax(cells_ap, tag):
        ct = pool.tile([NP112, C7, 64], f32, tag="ca_ct")
        nc.sync.dma_start(ct[:], cells_ap.rearrange("p (a b) -> p a b", a=C7))
        mx = pool.tile([NP112, C7], f32, tag=f"mx_{tag}")
        V.tensor_reduce(mx[:], ct[:], axis=AX.X, op=AluOp.max)
        eq = pool.tile([NP112, C7, 64], f32, tag="ca_eq")
        V.tensor_tensor(eq[:], ct[:], mx[:].unsqueeze(2).broadcast_to((NP112, C7, 64)),
                        AluOp.is_equal)
        sc = pool.tile([NP112, C7, 64], f32, tag="ca_sc")
        V.tensor_tensor(sc[:], eq[:], rev64[:].unsqueeze(1).broadcast_to((NP112, C7, 64)),
                        AluOp.mult)
        rs = pool.tile([NP112, C7], f32, tag=f"rs_{tag}")
        V.tensor_reduce(rs[:], sc[:], axis=AX.X, op=AluOp.max)
        jf = pool.tile([NP112, C7], f32, tag=f"jf_{tag}")
        V.tensor_scalar(jf[:], rs[:], 64.0, -1.0, AluOp.subtract, AluOp.mult)
        ji = pool.tile([NP112, C7], i32, tag=f"ji_{tag}")
        V.tensor_copy(ji[:], jf[:])
        # decode: ii=j>>3, jj=j&7; colcoord = 16+8*(7*(p&3)+c)+jj ; rowcoord = 16+8*(p>>2)+ii
        ii = pool.tile([NP112, C7], i32, tag=f"ii_{tag}")
        V.tensor_scalar(ii[:], ji[:], 3, None, AluOp.arith_shift_right)
        jj = pool.tile([NP112, C7], i32, tag=f"jj_{tag}")
        V.tensor_scalar(jj[:], ji[:], 7, None, AluOp.bitwise_and)
        cy = pool.tile([NP112, 1], i32, tag=f"cy_{tag}")
        V.tensor_scalar(cy[:], pidx112[:], 2, None, AluOp.arith_shift_right)
        cxh = pool.tile([NP112, 1], i32, tag=f"cxh_{tag}")
        V.tensor_scalar(cxh[:], pidx112[:], 3, None, AluOp.bitwise_and)
        # rowc = 16 + 8*cy + ii
        rowc = pool.tile([NP112, C7], i32, tag=f"rowc_{tag}")
        t1 = pool.tile([NP112, 1], i32, tag=f"t1_{tag}")
        V.tensor_scalar(t1[:], cy[:], 8, 16, AluOp.mult, AluOp.add)
        V.tensor_tensor(rowc[:], ii[:], t1[:].broadcast_to((NP112, C7)), AluOp.add)
        # colc = 16 + 8*(7*cxh + c) + jj = 16 + 56*cxh + 8*c + jj
        colc = pool.tile([NP112, C7], i32, tag=f"colc_{tag}")
        t2 = pool.tile([NP112, 1], i32, tag=f"t2_{tag}")
        V.tensor_scalar(t2[:], cxh[:], 56, 16, AluOp.mult, AluOp.add)
        t3 = pool.tile([NP112, C7], i32, tag=f"t3_{tag}")
        V.tensor_scalar(t3[:], ci7[:], 8, None, AluOp.mult)
        V.tensor_tensor(t3[:], t3[:], t2[:].broadcast_to((NP112, C7)), AluOp.add)
        V.tensor_tensor(colc[:], jj[:], t3[:], AluOp.add)
        # linear = colc*256 + rowc  (des[., colcoord, rowcoord] per reference swap)
        lin = pool.tile([NP112, C7], i32, tag=f"lin_{tag}")
        V.tensor_scalar(lin[:], colc[:], 256, None, AluOp.mult)
        V.tensor_tensor(lin[:], lin[:], rowc[:], AluOp.add)
        return jf, rowc, colc, lin

    # ================= det2 / distractor path (emit first: feeds collective) ====
    _, rowc2, colc2, lin2 = cell_argmax(din["det2c"], "d2")
    pidx2 = pool.tile([NP112, C7], i32)
    V.tensor_scalar(pidx2[:], lin2[:], 1, None, AluOp.arith_shift_right)
    pidx2_16 = pool.tile([NP112, C7], i16)
    V.tensor_copy(pidx2_16[:], pidx2[:])
    par2 = pool.tile([NP112, C7], i32)
    V.tensor_scalar(par2[:], lin2[:], 1, None, AluOp.bitwise_and)
    par2f = pool.tile([NP112, C7], f32)
    V.tensor_copy(par2f[:], par2[:])
    nc.sync.dma_start(bass.AP(sg_pidx2.tensor, 0, [[C7, NP112], [1, C7]]), pidx2_16[:])
    nc.sync.dma_start(bass.AP(sg_par2.tensor, 0, [[C7, NP112], [1, C7]]), par2f[:])
    # xd (row-decoded) and yd (col-decoded) rows for the distance mask
    rowc2f = pool.tile([NP112, C7], f32)
    V.tensor_copy(rowc2f[:], rowc2[:])
    colc2f = pool.tile([NP112, C7], f32)
    V.tensor_copy(colc2f[:], colc2[:])
    nc.sync.dma_start(bass.AP(sg_xyd.tensor, 0, [[C7, NP112], [1, C7]]), rowc2f[:])
    nc.sync.dma_start(bass.AP(sg_xyd.tensor, SLOTS, [[C7, NP112], [1, C7]]), colc2f[:])

    # wrapped idx readback + pair gather + parity select
    idx2w = pool.tile([128, 49], i16)
    for k in range(8):
        nc.sync.dma_start(idx2w[16 * k:16 * (k + 1), :],
                          bass.AP(sg_pidx2.tensor, 0, [[1, 16], [16, 49]]))
    par2G = pool.tile([128, C7], f32)
    nc.sync.dma_start(par2G[:], bass.AP(sg_par2.tensor, 0, [[1, 128], [128, C7]]))
    par2inv = pool.tile([128, C7], f32)
    V.tensor_scalar(par2inv[:], par2G[:], -1.0, 1.0, AluOp.mult, AluOp.add)
    dpair = pool.tile([128, C7, PAIR], f32, tag="pairg")
    G.dma_gather(dpair[:], din["desT2p"], idx2w[:], num_idxs=NPT,
                 num_idxs_reg=NPT, elem_size=PAIR)
    dexact = pool.tile([128, C7, D], f32)
    tlo = pool.tile([128, C7, D], f32, tag="seltmp")
    V.tensor_tensor(tlo[:], dpair[:, :, 0:D],
                    par2inv[:].unsqueeze(2).broadcast_to((128, C7, D)), AluOp.mult)
    thi = pool.tile([128, C7, D], f32, tag="seltmp2")
    V.tensor_tensor(thi[:], dpair[:, :, D:PAIR],
                    par2G[:].unsqueeze(2).broadcast_to((128, C7, D)), AluOp.mult)
    V.tensor_tensor(dexact[:], tlo[:], thi[:], AluOp.add)
    # transpose to [desc, slot] and ship own block to the collective
    distrT = pool.tile([128, SLOTS], f32)
    for ci in range(C7):
        pt = pspool2.tile([128, 128], f32, tag="tpose")
        nc.tensor.transpose(pt[:], dexact[:, ci, :], ident[:])
        S.copy(distrT[:, 128 * ci:128 * (ci + 1)], pt[:])
    nc.sync.dma_start(bass.AP(distr_in.tensor, 0, [[NPT, 128], [1, NPT]]),
                      distrT[:, 0:NPT])
    G.collective_compute("AllGather", AluOp.bypass,
                         replica_groups=[list(range(B))],
                         ins=[distr_in], outs=[distr_all])

    # ================= det1 / point path =================
    jf1, rowc1, colc1, lin1 = cell_argmax(din["det1c"], "d1")
    pidx1 = pool.tile([NP112, C7], i32)
    V.tensor_scalar(pidx1[:], lin1[:], 1, None, AluOp.arith_shift_right)
    pidx1_16 = pool.tile([NP112, C7], i16)
    V.tensor_copy(pidx1_16[:], pidx1[:])
    par1 = pool.tile([NP112, C7], i32)
    V.tensor_scalar(par1[:], lin1[:], 1, None, AluOp.bitwise_and)
    par1f = pool.tile([NP112, C7], f32)
    V.tensor_copy(par1f[:], par1[:])
    nc.sync.dma_start(bass.AP(sg_pidx1.tensor, 0, [[C7, NP112], [1, C7]]), pidx1_16[:])
    nc.sync.dma_start(bass.AP(sg_par1.tensor, 0, [[C7, NP112], [1, C7]]), par1f[:])

    # stage the 64-float block index + full linear index of the sample point
    blk1 = pool.tile([NP112, C7], i32)
    V.tensor_scalar(blk1[:], lin1[:], 6, None, AluOp.arith_shift_right)
    blk1_16 = pool.tile([NP112, C7], i16)
    V.tensor_copy(blk1_16[:], blk1[:])
    nc.sync.dma_start(bass.AP(sg_blk1.tensor, 0, [[C7, NP112], [1, C7]]), blk1_16[:])
    nc.sync.dma_start(bass.AP(sg_lin1.tensor, 0, [[C7, NP112], [1, C7]]), lin1[:])

    # G-layout: gather aflow-x/y + qlt1 blocks at lin1 and select elem lin1&63
    blk1w = pool.tile([128, 49], i16)
    for k in range(8):
        nc.sync.dma_start(blk1w[16 * k:16 * (k + 1), :],
                          bass.AP(sg_blk1.tensor, 0, [[1, 16], [16, 49]]))
    lin1G = pool.tile([128, C7], i32)
    nc.sync.dma_start(lin1G[:], bass.AP(sg_lin1.tensor, 0, [[1, 128], [128, C7]]))
    off1 = pool.tile([128, C7], i32)
    V.tensor_scalar(off1[:], lin1G[:], 63, None, AluOp.bitwise_and)
    off1f = pool.tile([128, C7], f32)
    V.tensor_copy(off1f[:], off1[:])
    ohg = pool.tile([128, C7, 64], f32)
    V.tensor_tensor(ohg[:], iot64g[:].unsqueeze(1).broadcast_to((128, C7, 64)),
                    off1f[:].unsqueeze(2).broadcast_to((128, C7, 64)), AluOp.is_equal)

    aqt = pool.tile([128, C7, 3, 64], f32, tag="pairg")
    G.dma_gather(aqt[:].rearrange("p c a b -> p c (a b)"), din["aq"], blk1w[:],
                 num_idxs=NPT, num_idxs_reg=NPT, elem_size=192)
    V.tensor_tensor(aqt[:], aqt[:], ohg[:].unsqueeze(2).broadcast_to((128, C7, 3, 64)),
                    AluOp.mult)
    aqv = pool.tile([128, C7, 3], f32)
    V.tensor_reduce(aqv[:], aqt[:], axis=AX.X, op=AluOp.add)
    x2f_raw = aqv[:, :, 0]
    y2f_raw = aqv[:, :, 1]
    q1G = aqv[:, :, 2]  # AP slices (not tiles)

    # xy2 = trunc(aflow + 0.5). The f32->i32 cast rounding mode differs between
    # HW (round-nearest-even) and CoreSim (trunc); build trunc explicitly in a
    # mode-agnostic way: floor = i0 - (float(i0) > f); trunc = floor + (f<0 & f!=floor)
    def trunc_to_int(raw_ap, tag):
        f = pool.tile([128, C7], f32, tag=f"tr_f_{tag}")
        V.tensor_scalar(f[:], raw_ap, 0.5, None, AluOp.add)
        i0 = pool.tile([128, C7], i32, tag=f"tr_i0_{tag}")
        V.tensor_copy(i0[:], f[:])
        fi = pool.tile([128, C7], f32, tag=f"tr_fi_{tag}")
        V.tensor_copy(fi[:], i0[:])
        gt = pool.tile([128, C7], f32, tag=f"tr_gt_{tag}")
        V.tensor_tensor(gt[:], fi[:], f[:], AluOp.is_gt)
        gti = pool.tile([128, C7], i32, tag=f"tr_gti_{tag}")
        V.tensor_copy(gti[:], gt[:])
        fl = pool.tile([128, C7], i32, tag=f"tr_fl_{tag}")
        V.tensor_tensor(fl[:], i0[:], gti[:], AluOp.subtract)
        flf = pool.tile([128, C7], f32, tag=f"tr_flf_{tag}")
        V.tensor_copy(flf[:], fl[:])
        neg = pool.tile([128, C7], f32, tag=f"tr_neg_{tag}")
        V.tensor_scalar(neg[:], f[:], 0.0, None, AluOp.is_lt)
        ne = pool.tile([128, C7], f32, tag=f"tr_ne_{tag}")
        V.tensor_tensor(ne[:], flf[:], f[:], AluOp.not_equal)
        V.tensor_tensor(neg[:], neg[:], ne[:], AluOp.mult)
        negi = pool.tile([128, C7], i32, tag=f"tr_negi_{tag}")
        V.tensor_copy(negi[:], neg[:])
        out = pool.tile([128, C7], i32, tag=f"tr_out_{tag}")
        V.tensor_tensor(out[:], fl[:], negi[:], AluOp.add)
        return out

    x2G = trunc_to_int(x2f_raw, "x")
    y2G = trunc_to_int(y2f_raw, "y")

    # validity mask (G-layout; write with c-split to stay in [0,784))
    m1 = pool.tile([128, C7], i32, tag="m1")
    m2 = pool.tile([128, C7], i32, tag="m2")
    V.tensor_scalar(m1[:], x2G[:], 0, None, AluOp.is_ge)
    V.tensor_scalar(m2[:], x2G[:], W, None, AluOp.is_lt)
    V.tensor_tensor(m1[:], m1[:], m2[:], AluOp.mult)
    V.tensor_scalar(m2[:], y2G[:], 0, None, AluOp.is_ge)
    V.tensor_tensor(m1[:], m1[:], m2[:], AluOp.mult)
    V.tensor_scalar(m2[:], y2G[:], H, None, AluOp.is_lt)
    V.tensor_tensor(m1[:], m1[:], m2[:], AluOp.mult)
    nc.sync.dma_start(bass.AP(mask_o.tensor, 0, [[1, 128], [128, 6]]), m1[:, 0:6])
    nc.sync.dma_start(bass.AP(mask_o.tensor, 768, [[1, 16], [1, 1]]), m1[0:16, 6:7])

    # ============ G-layout readbacks ============
    par1G = pool.tile([128, C7], f32)
    nc.sync.dma_start(par1G[:], bass.AP(sg_par1.tensor, 0, [[1, 128], [128, C7]]))
    par1inv = pool.tile([128, C7], f32)
    V.tensor_scalar(par1inv[:], par1G[:], -1.0, 1.0, AluOp.mult, AluOp.add)
    idx1w = pool.tile([128, 49], i16)
    for k in range(8):
        nc.sync.dma_start(idx1w[16 * k:16 * (k + 1), :],
                          bass.AP(sg_pidx1.tensor, 0, [[1, 16], [16, 49]]))
    xdR = pool.tile([128, NPT], f32)
    nc.sync.dma_start(xdR[:], bass.AP(sg_xyd.tensor, 0, [[0, 128], [1, NPT]]))
    ydR = pool.tile([128, NPT], f32)
    nc.sync.dma_start(ydR[:], bass.AP(sg_xyd.tensor, SLOTS, [[0, 128], [1, NPT]]))

    # per-offset clamped coords, parity, pair index (all in G-layout)
    xc = pool.tile([128, C7, NOFF], i32)
    V.tensor_tensor(xc[:], x2G[:].unsqueeze(2).broadcast_to((128, C7, NOFF)),
                    dxg[:].unsqueeze(1).broadcast_to((128, C7, NOFF)), AluOp.add)
    V.tensor_scalar(xc[:], xc[:], 0, W - 1, AluOp.max, AluOp.min)
    yc = pool.tile([128, C7, NOFF], i32)
    V.tensor_tensor(yc[:], y2G[:].unsqueeze(2).broadcast_to((128, C7, NOFF)),
                    dyg[:].unsqueeze(1).broadcast_to((128, C7, NOFF)), AluOp.add)
    V.tensor_scalar(yc[:], yc[:], 0, H - 1, AluOp.max, AluOp.min)
    parO = pool.tile([128, C7, NOFF], f32)
    tpo = pool.tile([128, C7, NOFF], i32, tag="tpo")
    V.tensor_scalar(tpo[:], xc[:], 1, None, AluOp.bitwise_and)
    V.tensor_copy(parO[:], tpo[:])
    parOinv = pool.tile([128, C7, NOFF], f32)
    V.tensor_scalar(parOinv[:], parO[:], -1.0, 1.0, AluOp.mult, AluOp.add)
    pidxO = pool.tile([128, C7, NOFF], i32)
    V.tensor_scalar(pidxO[:], xc[:], 1, None, AluOp.arith_shift_right)
    tyo = pool.tile([128, C7, NOFF], i32, tag="tyo")
    V.tensor_scalar(tyo[:], yc[:], W // 2, None, AluOp.mult)
    V.tensor_tensor(pidxO[:], pidxO[:], tyo[:], AluOp.add)
    V.tensor_scalar(pidxO[:], pidxO[:], 0, KP - 1, AluOp.max, AluOp.min)
    # gather idx lists: 7 pos row-segment starts + 12 neg pair indices
    gidx = pool.tile([128, NGL, C7], i32)
    k0all = pool.tile([128, NROWS, C7], i32)   # row segment start pairs
    for ri, (dy, r, seg, c0, nw) in enumerate(ROWS):
        ycr = pool.tile([128, C7], i32, tag="ycr")
        V.tensor_scalar(ycr[:], y2G[:], dy, None, AluOp.add)
        V.tensor_scalar(ycr[:], ycr[:], 0, H - 1, AluOp.max, AluOp.min)
        k0 = pool.tile([128, C7], i32, tag="k0r")
        V.tensor_scalar(k0[:], x2G[:], r, None, AluOp.subtract)
        V.tensor_scalar(k0[:], k0[:], 1, None, AluOp.arith_shift_right)
        V.tensor_scalar(k0[:], k0[:], 0, W // 2 - seg, AluOp.max, AluOp.min)
        V.tensor_copy(k0all[:, ri, :], k0[:])
        V.tensor_scalar(ycr[:], ycr[:], W // 2, None, AluOp.mult)
        V.tensor_tensor(gidx[:, ri, :], ycr[:], k0[:], AluOp.add)
    V.tensor_copy(gidx[:, NROWS:NGL, :],
                  pidxO[:, :, NPOS:NOFF].rearrange("p c o -> p o c"))
    gidx16 = pool.tile([128, NGL, C7], i16)
    V.tensor_copy(gidx16[:], gidx[:])
    nc.sync.dma_start(bass.AP(sg_goff.tensor, 0,
                              [[1, 128], [SLOTS, NGL], [128, C7]]), gidx16[:])
    goffw = pool.tile([128, NGL, 56], i16)
    for k in range(8):
        nc.sync.dma_start(goffw[16 * k:16 * (k + 1), :, :],
                          bass.AP(sg_goff.tensor, 0, [[1, 16], [SLOTS, NGL], [16, 56]]))

    # s_des1 gather + parity select
    spair = pool.tile([128, C7, PAIR], f32, tag="pairg")
    G.dma_gather(spair[:], din["desT1p"], idx1w[:], num_idxs=NPT,
                 num_idxs_reg=NPT, elem_size=PAIR)
    sexact = pool.tile([128, C7, D], f32)
    tl1 = pool.tile([128, C7, D], f32, tag="seltmp")
    V.tensor_tensor(tl1[:], spair[:, :, 0:D],
                    par1inv[:].unsqueeze(2).broadcast_to((128, C7, D)), AluOp.mult)
    th1 = pool.tile([128, C7, D], f32, tag="seltmp2")
    V.tensor_tensor(th1[:], spair[:, :, D:PAIR],
                    par1G[:].unsqueeze(2).broadcast_to((128, C7, D)), AluOp.mult)
    V.tensor_tensor(sexact[:], tl1[:], th1[:], AluOp.add)

    # ============ sT transposes ============
    sT = pool.tile([128, SLOTS], f32)
    for ci in range(C7):
        pt = pspool2.tile([128, 128], f32, tag="tpose")
        nc.tensor.transpose(pt[:], sexact[:, ci, :], ident[:])
        S.copy(sT[:, 128 * ci:128 * (ci + 1)], pt[:])

    # ============ distractor GEMM (global, unmasked) ============
    CH = 512
    for bb in range(B):
        rhs = pool.tile([128, NPT], f32, tag="rhs")
        nc.sync.dma_start(rhs[:], bass.AP(distr_all.tensor, bb * D * NPT,
                                          [[NPT, 128], [1, NPT]]))
        for ci in range(C7):
            rows = 128 if ci < 6 else 16
            for ch in range(2):
                c0 = ch * CH
                cw = min(CH, NPT - c0)
                pm = pspool.tile([128, CH], f32, tag="gemm")
                nc.tensor.matmul(pm[:, 0:cw], sT[:, 128 * ci:128 * (ci + 1)],
                                 rhs[:, c0:c0 + cw], start=True, stop=True)
                ob = opool.tile([128, CH], f32, tag="ochunk")
                S.copy(ob[:, 0:cw], pm[:, 0:cw])
                nc.sync.dma_start(
                    bass.AP(scores_o.tensor,
                            128 * ci * NCOL + 1 + NNEG + bb * NPT + c0,
                            [[NCOL, rows], [1, cw]]), ob[0:rows, 0:cw])

    # ============ own-block GEMM + distance mask (static; host overlays) ====
    for ci in range(C7):
        rows = 128 if ci < 6 else 16
        # dis2 = (xd - x2)^2 + (yd - y2)^2 over own 784 cols
        x2s = pool.tile([128, 1], f32, tag="x2s")
        V.tensor_copy(x2s[:], x2G[:, ci:ci + 1])
        y2s = pool.tile([128, 1], f32, tag="y2s")
        V.tensor_copy(y2s[:], y2G[:, ci:ci + 1])
        dxm = pool.tile([128, NPT], f32, tag="dxm")
        V.tensor_tensor(dxm[:], xdR[:], x2s[:].broadcast_to((128, NPT)), AluOp.subtract)
        V.tensor_tensor(dxm[:], dxm[:], dxm[:], AluOp.mult)
        dym = pool.tile([128, NPT], f32, tag="dym")
        V.tensor_tensor(dym[:], ydR[:], y2s[:].broadcast_to((128, NPT)), AluOp.subtract)
        V.tensor_tensor(dym[:], dym[:], dym[:], AluOp.mult)
        V.tensor_tensor(dxm[:], dxm[:], dym[:], AluOp.add)
        keep = pool.tile([128, NPT], f32, tag="keep")
        V.tensor_scalar(keep[:], dxm[:], float(POS_R ** 2), None, AluOp.is_ge)
        for ch in range(2):
            c0 = ch * CH
            cw = min(CH, NPT - c0)
            pm = pspool.tile([128, CH], f32, tag="gemm")
            nc.tensor.matmul(pm[:, 0:cw], sT[:, 128 * ci:128 * (ci + 1)],
                             distrT[:, c0:c0 + cw], start=True, stop=True)
            ob = opool.tile([128, CH], f32, tag="ochunk")
            S.copy(ob[:, 0:cw], pm[:, 0:cw])
            V.tensor_tensor(ob[:, 0:cw], ob[:, 0:cw], keep[:, c0:c0 + cw], AluOp.mult)
            nc.sync.dma_start(
                bass.AP(ownblk_o.tensor, 128 * ci * NPT + c0, [[NPT, rows], [1, cw]]),
                ob[0:rows, 0:cw])

    # ============ dot computation ============
    dotsAll = pool.tile([128, C7, NOFF], f32)
    # pos rows: gather SEG-pair segments, per-pixel dots, select by position
    for ri, (dy, r, seg, c0, nw) in enumerate(ROWS):
        npx = 2 * seg
        gp = gpool.tile([128, C7, seg * PAIR], f32, tag="gseg")
        seg_src = bass.AP(din["desT2p"].tensor, 0,
                          [[PAIR, KP - seg + 1], [1, seg * PAIR]])
        G.dma_gather(gp[:], seg_src, goffw[:, ri, 0:49], num_idxs=NPT,
                     num_idxs_reg=NPT, elem_size=seg * PAIR, elem_step=PAIR)
        pr = ppool.tile([128, C7, npx, D], f32, tag="pr")
        V.tensor_tensor(pr[:], gp[:].rearrange("p c (h d) -> p c h d", h=npx),
                        sexact[:].unsqueeze(2).broadcast_to((128, C7, npx, D)),
                        AluOp.mult)
        pxd = pool.tile([128, C7, npx], f32, tag="pxd")
        V.tensor_reduce(pxd[:], pr[:], axis=AX.X, op=AluOp.add)
        # position of each offset's pixel inside the segment: xc - 2*k0
        posr = pool.tile([128, C7, nw], f32, tag="posr")
        k2 = pool.tile([128, C7], i32, tag="k2r")
        V.tensor_scalar(k2[:], k0all[:, ri, :], 2, None, AluOp.mult)
        posi = pool.tile([128, C7, nw], i32, tag="posi")
        V.tensor_tensor(posi[:], xc[:, :, c0:c0 + nw],
                        k2[:].unsqueeze(2).broadcast_to((128, C7, nw)), AluOp.subtract)
        V.tensor_copy(posr[:], posi[:])
        iopx = pool.tile([128, npx], f32, tag="iopx")
        G.iota(iopx[:], pattern=[[1, npx]], base=0, channel_multiplier=0,
               allow_small_or_imprecise_dtypes=True)
        ohr = ppool.tile([128, C7, nw, npx], f32, tag="ohr")
        V.tensor_tensor(ohr[:],
                        iopx[:].unsqueeze(1).unsqueeze(2).broadcast_to((128, C7, nw, npx)),
                        posr[:].unsqueeze(3).broadcast_to((128, C7, nw, npx)),
                        AluOp.is_equal)
        V.tensor_tensor(ohr[:], ohr[:],
                        pxd[:].unsqueeze(2).broadcast_to((128, C7, nw, npx)), AluOp.mult)
        V.tensor_reduce(dotsAll[:, :, c0:c0 + nw], ohr[:], axis=AX.X, op=AluOp.add)
    # neg offsets: per-offset pair gathers + parity blend
    dots2All = pool.tile([128, C7, NNEG, 2], f32)
    for o in range(NNEG):
        gp = gpool.tile([128, C7, PAIR], f32, tag="gseg")
        G.dma_gather(gp[:], din["desT2p"], goffw[:, NROWS + o, 0:49], num_idxs=NPT,
                     num_idxs_reg=NPT, elem_size=PAIR)
        pr = ppool.tile([128, C7, 2, D], f32, tag="pr")
        V.tensor_tensor(pr[:], gp[:].rearrange("p c (h d) -> p c h d", h=2),
                        sexact[:].unsqueeze(2).broadcast_to((128, C7, 2, D)),
                        AluOp.mult)
        V.tensor_reduce(dots2All[:, :, o, :], pr[:], axis=AX.X, op=AluOp.add)
    b0 = pool.tile([128, C7, NNEG], f32, tag="blend0")
    V.tensor_tensor(b0[:], dots2All[:, :, :, 0], parOinv[:, :, NPOS:NOFF], AluOp.mult)
    b1 = pool.tile([128, C7, NNEG], f32, tag="blend1")
    V.tensor_tensor(b1[:], dots2All[:, :, :, 1], parO[:, :, NPOS:NOFF], AluOp.mult)
    V.tensor_tensor(dotsAll[:, :, NPOS:NOFF], b0[:], b1[:], AluOp.add)

    # pscores / first-argmax over the 29 positive offsets
    psc = pool.tile([128, C7], f32)
    V.tensor_reduce(psc[:], dotsAll[:, :, 0:NPOS], axis=AX.X, op=AluOp.max)
    eqp = pool.tile([128, C7, NPOS], f32)
    V.tensor_tensor(eqp[:], dotsAll[:, :, 0:NPOS],
                    psc[:].unsqueeze(2).broadcast_to((128, C7, NPOS)), AluOp.is_equal)
    scp = pool.tile([128, C7, NPOS], f32)
    V.tensor_tensor(scp[:], eqp[:], revP[:].unsqueeze(1).broadcast_to((128, C7, NPOS)),
                    AluOp.mult)
    rsp = pool.tile([128, C7], f32)
    V.tensor_reduce(rsp[:], scp[:], axis=AX.X, op=AluOp.max)
    jsel = pool.tile([128, C7], f32)
    V.tensor_scalar(jsel[:], rsp[:], float(NPOS), -1.0, AluOp.subtract, AluOp.mult)
    ohp = pool.tile([128, C7, NPOS], f32)
    V.tensor_tensor(ohp[:], iotP[:].unsqueeze(1).broadcast_to((128, C7, NPOS)),
                    jsel[:].unsqueeze(2).broadcast_to((128, C7, NPOS)), AluOp.is_equal)
    xcf = pool.tile([128, C7, NPOS], f32)
    V.tensor_copy(xcf[:], xc[:, :, 0:NPOS])
    ycf = pool.tile([128, C7, NPOS], f32)
    V.tensor_copy(ycf[:], yc[:, :, 0:NPOS])
    tsel = pool.tile([128, C7, NPOS], f32, tag="tsel")
    V.tensor_tensor(tsel[:], ohp[:], xcf[:], AluOp.mult)
    selx = pool.tile([128, C7], f32)
    V.tensor_reduce(selx[:], tsel[:], axis=AX.X, op=AluOp.add)
    V.tensor_tensor(tsel[:], ohp[:], ycf[:], AluOp.mult)
    sely = pool.tile([128, C7], f32)
    V.tensor_reduce(sely[:], tsel[:], axis=AX.X, op=AluOp.add)
    # qlt2 block gather: linear = sely*W + selx ; block = lin>>6 ; off = lin&63
    slin = pool.tile([128, C7], f32, tag="slin")
    V.tensor_scalar(slin[:], sely[:], float(W), None, AluOp.mult)
    V.tensor_tensor(slin[:], slin[:], selx[:], AluOp.add)
    slini = pool.tile([128, C7], i32)
    V.tensor_copy(slini[:], slin[:])
    qblk = pool.tile([128, C7], i32)
    V.tensor_scalar(qblk[:], slini[:], 6, None, AluOp.arith_shift_right)
    V.tensor_scalar(qblk[:], qblk[:], 0, 1023, AluOp.max, AluOp.min)
    qoff = pool.tile([128, C7], i32)
    V.tensor_scalar(qoff[:], slini[:], 63, None, AluOp.bitwise_and)
    qofff = pool.tile([128, C7], f32)
    V.tensor_copy(qofff[:], qoff[:])
    qblk16 = pool.tile([128, C7], i16)
    V.tensor_copy(qblk16[:], qblk[:])
    nc.sync.dma_start(bass.AP(sg_qblk.tensor, 0, [[1, 128], [128, C7]]), qblk16[:])
    qblkw = pool.tile([128, 49], i16)
    for k in range(8):
        nc.sync.dma_start(qblkw[16 * k:16 * (k + 1), :],
                          bass.AP(sg_qblk.tensor, 0, [[1, 16], [16, 49]]))
    q2t = pool.tile([128, C7, 64], f32)
    G.dma_gather(q2t[:], din["qlt2f"], qblkw[:], num_idxs=NPT,
                 num_idxs_reg=NPT, elem_size=64)
    oh64 = pool.tile([128, C7, 64], f32)
    V.tensor_tensor(oh64[:], iot64g[:].unsqueeze(1).broadcast_to((128, C7, 64)),
                    qofff[:].unsqueeze(2).broadcast_to((128, C7, 64)), AluOp.is_equal)
    V.tensor_tensor(q2t[:], q2t[:], oh64[:], AluOp.mult)
    q2v = pool.tile([128, C7], f32)
    V.tensor_reduce(q2v[:], q2t[:], axis=AX.X, op=AluOp.add)
    qfin = pool.tile([128, C7], f32)
    V.tensor_tensor(qfin[:], q1G, q2v[:], AluOp.add)
    V.tensor_scalar(qfin[:], qfin[:], 0.5, None, AluOp.mult)
    # outputs: qlt, pscores, nscores (split c=0..5 full, c=6 first 16 rows)
    nc.sync.dma_start(bass.AP(qlt_o.tensor, 0, [[1, 128], [128, 6]]), qfin[:, 0:6])
    nc.sync.dma_start(bass.AP(qlt_o.tensor, 768, [[1, 16], [1, 1]]), qfin[0:16, 6:7])
    nc.sync.dma_start(bass.AP(scores_o.tensor, 0, [[NCOL, 128], [128 * NCOL, 6]]),
                      psc[:, 0:6])
    nc.sync.dma_start(bass.AP(scores_o.tensor, 768 * NCOL, [[NCOL, 16], [1, 1]]),
                      psc[0:16, 6:7])
    nc.sync.dma_start(bass.AP(scores_o.tensor, 1, [[NCOL, 128], [128 * NCOL, 6], [1, NNEG]]),
                      dotsAll[:, 0:6, NPOS:NOFF])
    nc.sync.dma_start(bass.AP(scores_o.tensor, 768 * NCOL + 1, [[NCOL, 16], [1, NNEG]]),
                      dotsAll[0:16, 6, NPOS:NOFF])

    ctx.close()


# ---------------- host side ----------------
_NC = None

def _get_nc():
    global _NC
    if _NC is None:
        _NC = _build_program()
    return _NC


def _cells(img):
    # [256,256] -> [112, 448] cell layout: p=4*cy+cxhi, f=cxlo*64+ii*8+jj
    a = img[BORD:H - BORD, BORD:W - BORD]
    a = a.reshape(HC, CELL, HC, CELL).transpose(0, 2, 1, 3)      # [cy, cx, ii, jj]
    a = a.reshape(HC, 4, C7, CELL, CELL)                          # [cy, cxhi, cxlo, ii, jj]
    return np.ascontiguousarray(a.reshape(NP112, C7 * 64))


def _make_in_maps(des1, det1, qlt1, des2, det2, qlt2, aflow):
    ctab = np.zeros(2 * 64, np.int32)
    ctab[0:NOFF] = DX
    ctab[64:64 + NOFF] = DY
    posj = np.zeros(64, np.float32)
    posj[0:NPOS] = REFJ.astype(np.float32)
    posj[32:32 + NPOS] = float(NPOS) - REFJ.astype(np.float32)
    in_maps = []
    for b in range(B):
        d1 = np.ascontiguousarray(des1[b].transpose(1, 2, 0)).reshape(KP, PAIR)
        d2 = np.ascontiguousarray(des2[b].transpose(1, 2, 0)).reshape(KP, PAIR)
        in_maps.append({
            "desT1p": d1,
            "desT2p": d2,
            "det1c": _cells(det1[b, 0]),
            "det2c": _cells(det2[b, 0]),
            "aq": np.ascontiguousarray(np.concatenate(
                [aflow[b, 0].reshape(1024, 64), aflow[b, 1].reshape(1024, 64),
                 qlt1[b, 0].reshape(1024, 64)], axis=1)),
            "posj": posj,
            "qlt2f": np.ascontiguousarray(qlt2[b, 0].reshape(1024, 64)),
            "ctab": ctab,
        })
    return in_maps


def _assemble(results):
    scores = np.empty((ND, NCOL), np.float32)
    qlt = np.empty((ND, 1), np.float32)
    mask = np.empty((B, NPT), bool)
    for b in range(B):
        r = results[b]
        rows = slice(b * NPT, (b + 1) * NPT)
        scores[rows] = r["scores_o"]
        # overlay the masked own-batch distractor block
        scores[rows, 1 + NNEG + b * NPT: 1 + NNEG + (b + 1) * NPT] = r["ownblk_o"]
        qlt[rows, 0] = r["qlt_o"]
        mask[b] = r["mask_o"].astype(bool)
    labels = np.zeros((ND, NCOL), bool)
    labels[:, :1] = True
    return scores, labels, mask, qlt


def kernel(des1, det1, qlt1, des2, det2, qlt2, aflow):
    des1 = np.asarray(des1); det1 = np.asarray(det1); qlt1 = np.asarray(qlt1)
    des2 = np.asarray(des2); det2 = np.asarray(det2); qlt2 = np.asarray(qlt2)
    aflow = np.asarray(aflow)
    in_maps = _make_in_maps(des1, det1, qlt1, des2, det2, qlt2, aflow)
    nc = _get_nc()
    trace = os.environ.get("KERNEL_TRACE") == "1"
    res = run_bass_kernel_spmd(nc, in_maps, core_ids=list(range(B)), trace=trace)
    if trace:
        kernel.last_exec_ns = res.exec_time_ns
    return _assemble(res.results)


kernel.last_exec_ns = None
